# revision 60
# baseline (speedup 1.0000x reference)
"""KBLaM BitNet attention on 8 Trainium2 NeuronCores (tensor-parallel over heads).

Core c owns q-heads 4c..4c+3, kv-head c, kb heads 4c..4c+3, and the matching
input-dim slice of Wo. Each core returns a partial o_proj output in f16; the
host sums.

Numerics: BitLinear projections are exact (integer activations / ternary
weights in bf16, fp32 PSUM accumulation). Attention (QK^T, probs, PV) runs in
f16 with fp32 PSUM accumulation; RoPE runs in f16. The o_proj activation
quantization uses a round-half-even saturating int8 cast, identical to
clip(round(x*a), -128, 127). Two pipelined AllGathers (one per 512-token half)
provide the global per-token amax for that quantization.
"""
import sys
if "/opt/trn_rl_repo" not in sys.path:
    sys.path.insert(0, "/opt/trn_rl_repo")
import numpy as np
import ml_dtypes

import concourse.mybir as mybir
import concourse.tile as tile
from concourse import bacc
from concourse import bass_utils
from concourse.masks import make_identity

F32 = mybir.dt.float32
F16 = mybir.dt.float16
BF16 = mybir.dt.bfloat16
F8 = mybir.dt.float8e4
I8 = mybir.dt.int8
ALU = mybir.AluOpType
ACTF = mybir.ActivationFunctionType
AX = mybir.AxisListType

B, Q, H = 1, 1024, 2048
NH, NKV, HD = 32, 8, 64
KB = 2048
NCORES = 8
HPC = NH // NCORES            # 4 q heads per core
P = 128
TT = Q // P                   # 8 token tiles
TTH = TT // 2                 # 4 token tiles per half
KO = H // P                   # 16 hidden k-tiles
M1 = 5                        # proj out tiles: kbq 256 | q 256 | (k 64 + v 64)
NJT = KB // P                 # 16 kb j-tiles
G = 3                         # scores psum group size (j-tiles per exp)
SCALE = 0.125                 # 1/sqrt(HD)
KB_BIAS = float(np.log(4096.0) - np.log(float(KB)))

_CACHE = {}
_MARKS = []


def _grouped(blocks, g):
    return [blocks[i:i + g] for i in range(0, len(blocks), g)]


def _build(MASK_CFG, NM):
    """MASK_CFG: per tc in (0,1): (fulls tuple, maskeds tuple of (pjt, mi, c0, c1)).
    NM = total number of masked em blocks staged in em2."""
    nc = bacc.Bacc("TRN2", target_bir_lowering=False, debug=False, num_devices=NCORES)

    x_d = nc.dram_tensor("x", [Q, H], F32, kind="ExternalInput").ap()
    w1t_d = nc.dram_tensor("w1t", [P, KO, 640], F8, kind="ExternalInput").ap()
    wsvec_d = nc.dram_tensor("wsvec", [640], F32, kind="ExternalInput").ap()
    cos_d = nc.dram_tensor("cos2", [HD, Q], F16, kind="ExternalInput").ap()
    sin_d = nc.dram_tensor("sin2", [HD, Q], F16, kind="ExternalInput").ap()
    kbkt_d = nc.dram_tensor("kbkt", [HD, HPC, KB], F16, kind="ExternalInput").ap()
    kbv_d = nc.dram_tensor("kbv", [P, HPC, NJT, 65], F16, kind="ExternalInput").ap()
    em2_d = nc.dram_tensor("em2", [P, max(NM, 1), 512], F16, kind="ExternalInput").ap()
    rmat_d = nc.dram_tensor("rmat", [HD, HD], F16, kind="ExternalInput").ap()
    wot_d = nc.dram_tensor("wot", [HPC * HD, H], BF16, kind="ExternalInput").ap()
    osc_d = nc.dram_tensor("oscale", [P, 1], F32, kind="ExternalInput").ap()
    y_d = nc.dram_tensor("y", [Q, H], F16, kind="ExternalOutput").ap()

    def copy_(eng, out, in_):
        if eng is nc.scalar:
            nc.scalar.copy(out, in_)
        else:
            eng.tensor_copy(out, in_)

    def scale_(eng, out, in_, sc):
        if eng is nc.scalar:
            nc.scalar.mul(out, in_, sc)
        else:
            eng.tensor_scalar(out, in_, sc, None, ALU.mult)

    def mk(label):
        _MARKS.append((label, nc.next_id()))

    with tile.TileContext(nc) as tc_:
        with tc_.tile_pool(name="cst", bufs=1) as cst, \
             tc_.tile_pool(name="xp", bufs=3) as xp, \
             tc_.tile_pool(name="sc", bufs=2, space="PSUM") as scp, \
             tc_.tile_pool(name="pop", bufs=1, space="PSUM") as pop, \
             tc_.tile_pool(name="pap", bufs=1, space="PSUM") as pap, \
             tc_.tile_pool(name="wk", bufs=2) as wk, \
             tc_.tile_pool(name="wk3", bufs=3) as wk3, \
             tc_.tile_pool(name="wk4", bufs=4) as wk4, \
             tc_.tile_pool(name="dram", bufs=1, space="DRAM") as dram:

            # ---------------- DMA prologue (ordered for earliest use) -------
            xt = {}
            for tt in range(TTH):
                t = xp.tile([P, H], F32, tag="x")
                nc.sync.dma_start(t[:], x_d[tt * P:(tt + 1) * P, :])
                xt[tt] = t
            w1t = cst.tile([P, KO, 640], F8)
            nc.sync.dma_start(w1t[:], w1t_d)
            wspp = cst.tile([P, M1], F32)
            nc.sync.dma_start(wspp[:], wsvec_d.rearrange("(m p) -> p m", p=P))
            kbkt = cst.tile([HD, HPC, KB], F16)
            nc.sync.dma_start(kbkt[:], kbkt_d)
            kbv = cst.tile([P, HPC, NJT, 65], F16)
            nc.sync.dma_start(kbv[:], kbv_d)
            cos2 = cst.tile([HD, Q], F16)
            sin2 = cst.tile([HD, Q], F16)
            em2 = cst.tile([P, max(NM, 1), 512], F16)
            rmat = cst.tile([HD, HD], F16)

            # ---------------- resident constants / buffers ------------------
            kbias = cst.tile([P, 1], F32)
            nc.vector.memset(kbias[:], KB_BIAS)
            ident = cst.tile([P, P], BF16)
            make_identity(nc, ident)

            inv_a_cols = cst.tile([P, TT], F32)
            inv_a_dram = dram.tile([Q], F32)
            inv_ab = cst.tile([P, Q], F32)
            xqT = cst.tile([P, KO, Q], BF16)
            qT = cst.tile([HD, HPC, Q], F16)
            kbqT = cst.tile([HD, HPC, Q], F16)
            kT = cst.tile([HD, Q], F16)
            vTf = cst.tile([HD, Q], F16)
            v_sb = cst.tile([P, TT, 65], F16)
            nc.vector.memset(v_sb[:], 1.0)
            att = cst.tile([P, TT, HPC * HD], F32)
            g_loc = cst.tile([P, TT], F32)
            g_glob = cst.tile([P, TT], F32)
            cc_in = [dram.tile([P, TTH], F32, name=f"ci{c}") for c in range(2)]
            cc_out = [dram.tile([NCORES, P, TTH], F32, name=f"co{c}") for c in range(2)]

            # ---------------- phase helpers --------------------------------
            def phaseA_tile(tt, qeng, ceng, eeng):
                """quantize x tile tt, transpose into xqT."""
                x_ = xt[tt]
                m = wk.tile([P, 1], F32, tag="am")
                nc.vector.tensor_reduce(m[:], x_[:], AX.X, ALU.max,
                                        apply_absolute_value=True)
                nc.vector.tensor_scalar(m[:], m[:], 1e-5, None, ALU.max)
                rec = wk.tile([P, 1], F32, tag="arec")
                nc.vector.reciprocal(rec[:], m[:])
                a_col = wk.tile([P, 1], F32, tag="acol")
                nc.vector.tensor_scalar(a_col[:], rec[:], 127.0, None, ALU.mult)
                nc.vector.tensor_scalar(inv_a_cols[:, tt:tt + 1], m[:],
                                        1.0 / 127.0, None, ALU.mult)
                xi = wk.tile([P, H], I8, tag="axi")
                nc.vector.tensor_scalar(xi[:], x_[:], a_col[:], None, ALU.mult)
                xq = wk.tile([P, H], BF16, tag="axq")
                copy_(ceng, xq[:], xi[:])
                for g in range(2):
                    pt = pap.tile([P, 8, P], BF16, tag="tp")
                    for i in range(8):
                        ko = 8 * g + i
                        nc.tensor.transpose(pt[:, i, :],
                                            xq[:, ko * P:(ko + 1) * P], ident[:])
                    copy_(eeng, xqT[:, 8 * g:8 * g + 8, tt * P:(tt + 1) * P], pt[:])

            def inv_a_bounce(c):
                sl = slice(512 * c, 512 * (c + 1))
                nc.sync.dma_start(
                    inv_a_dram[:].rearrange("(o p) -> p o", p=P)[:, TTH * c:TTH * (c + 1)],
                    inv_a_cols[:, TTH * c:TTH * (c + 1)])
                nc.sync.dma_start(
                    inv_ab[:, sl],
                    inv_a_dram[sl].unsqueeze(0).partition_broadcast(P))

            b_slots = {}

            def phaseB_m1(m1, c, sub=None):
                """projection group m1 for token chunk c + dequant to f16.
                sub=0/1 runs a 256-token half (dequant after sub=1)."""
                sl = slice(512 * c, 512 * (c + 1))
                if sub is None:
                    s = scp.tile([P, G, 512], F32, tag="s")
                    ps = s[:, 0, :]
                    msl = sl
                else:
                    if sub == 0:
                        b_slots[(m1, c)] = scp.tile([P, G, 512], F32, tag="s",
                                                    name=f"bs{m1}_{c}")
                    s = b_slots[(m1, c)]
                    ps = s[:, 0, sub * 256:(sub + 1) * 256]
                    msl = slice(512 * c + sub * 256, 512 * c + (sub + 1) * 256)
                for ko in range(KO):
                    nc.tensor.matmul(ps, w1t[:, ko, m1 * P:(m1 + 1) * P],
                                     xqT[:, ko, msl],
                                     start=(ko == 0), stop=(ko == KO - 1))
                if sub == 0:
                    return
                ps = s[:, 0, :]
                if m1 < 2:      # kbq heads 2*m1, 2*m1+1 (no rope)
                    dsts = (kbqT[:, 2 * m1, sl], kbqT[:, 2 * m1 + 1, sl])
                elif m1 < 4:    # q heads (rope applied in-place later)
                    dsts = (qT[:, 2 * (m1 - 2), sl], qT[:, 2 * (m1 - 2) + 1, sl])
                else:           # k | v
                    dsts = (kT[:, sl], vTf[:, sl])
                nc.vector.scalar_tensor_tensor(
                    dsts[0], ps[:HD], wspp[:HD, m1:m1 + 1],
                    inv_ab[:HD, sl], ALU.mult, ALU.mult)
                nc.vector.scalar_tensor_tensor(
                    dsts[1], ps[HD:], wspp[HD:, m1:m1 + 1],
                    inv_ab[HD:, sl], ALU.mult, ALU.mult)

            def rope(dst, heads, c, tag):
                """in-place RoPE on dst[HD, (h,) 512c:512c+512], f16.
                heads=None: single-vector (k). rotate_half is a fixed 64x64
                signed permutation, computed on the PE (low latency)."""
                sl = slice(512 * c, 512 * (c + 1))
                hs = [0] if heads is None else heads
                nh = len(hs)
                dsl = [dst[:, sl].unsqueeze(1)[:, 0, :] if heads is None
                       else dst[:, h, sl] for h in hs]
                srot = []
                for i in range(nh):
                    if i % G == 0:
                        s = scp.tile([P, G, 512], F32, tag="s",
                                     name=f"rp{tag}{c}{i}")
                    srot.append(s[0:HD, i % G, :])
                    nc.tensor.matmul(srot[i], rmat[:], dsl[i],
                                     start=True, stop=True)
                t2 = wk.tile([HD, 2 if tag == "q" else 1, 512], F16,
                             tag=f"sw{tag}")
                t1 = wk.tile([HD, 2 if tag == "q" else 1, 512], F16,
                             tag=f"rt{tag}")
                for i in range(nh):
                    nc.vector.tensor_tensor(t2[:, i, :], srot[i],
                                            sin2[:, sl], ALU.mult)
                    nc.vector.tensor_tensor(t1[:, i, :], dsl[i],
                                            cos2[:, sl], ALU.mult)
                    nc.vector.tensor_tensor(dsl[i], t1[:, i, :], t2[:, i, :],
                                            ALU.add)

            def v_transpose(c):
                vt = wk.tile([P, TTH, HD], F16, tag="vt")
                nc.sync.dma_start_transpose(vt[:], vTf[:, 512 * c:512 * (c + 1)])
                nc.vector.tensor_copy(v_sb[:, TTH * c:TTH * (c + 1), 0:HD], vt[:])

            def attn_head(h, tc, mid_cb=None, post_cb=None):
                """one head, one 512-query chunk."""
                sl = slice(512 * tc, 512 * (tc + 1))
                kbq_s = kbqT[:, h, sl]
                q_s = qT[:, h, sl]
                po = pop.tile([P, 512], F32, tag="po")
                state = {"started": False}

                def pv(lhsT, pt_ap, c0, c1, last):
                    nc.tensor.matmul(po[0:65, c0:c1], lhsT, pt_ap,
                                     start=(not state["started"]), stop=last,
                                     skip_group_check=True)
                    state["started"] = True

                fulls, maskeds = MASK_CFG[tc]
                # bank-pack masked blocks: first-fit-decreasing into 512-wide
                # PSUM banks so one exp covers several partial blocks
                banks = []   # list of [used_width, [(pjt, mi, c0, c1, off)]]
                for (pjt, mi, c0, c1) in sorted(maskeds, key=lambda t: t[2] - t[3]):
                    w = c1 - c0
                    for b in banks:
                        if b[0] + w <= 512:
                            b[1].append((pjt, mi, c0, c1, b[0]))
                            b[0] += w
                            break
                    else:
                        banks.append([w, [(pjt, mi, c0, c1, 0)]])
                groups = [("kb", ch) for ch in _grouped(list(range(NJT)), G)]
                groups += [("pf", ch) for ch in _grouped(list(fulls), G)]
                groups += [("pm", ch) for ch in
                           _grouped([b[1] for b in banks], G)]
                npv = sum(sum(len(it) for it in g[1]) if g[0] == "pm"
                          else len(g[1]) for g in groups)
                pvi = 0
                nkb = len(_grouped(list(range(NJT)), G))
                for gi, (kind, items) in enumerate(groups):
                    if gi == nkb and mid_cb is not None:
                        mid_cb()
                    n = len(items)
                    s = scp.tile([P, G, 512], F32, tag="s")
                    pt = wk4.tile([P, G, 512], F16, tag="pt")
                    if kind == "kb":
                        for i, jt in enumerate(items):
                            nc.tensor.matmul(s[:, i, :],
                                             kbkt[:, h, jt * P:(jt + 1) * P],
                                             kbq_s, start=True, stop=True)
                        nc.scalar.activation(pt[:, 0:n, :], s[:, 0:n, :], ACTF.Exp,
                                             bias=kbias[:], scale=SCALE)
                        for i, jt in enumerate(items):
                            pvi += 1
                            pv(kbv[:, h, jt, :], pt[:, i, :], 0, 512, pvi == npv)
                    elif kind == "pf":
                        for i, pjt in enumerate(items):
                            nc.tensor.matmul(s[:, i, :],
                                             kT[:, pjt * P:(pjt + 1) * P],
                                             q_s, start=True, stop=True)
                        nc.scalar.activation(pt[:, 0:n, :], s[:, 0:n, :], ACTF.Exp,
                                             bias=0.0, scale=SCALE)
                        for i, pjt in enumerate(items):
                            pvi += 1
                            pv(v_sb[:, pjt, :], pt[:, i, :], 0, 512, pvi == npv)
                    else:  # masked: items = list of banks of placed blocks
                        for i, bank in enumerate(items):
                            for (pjt, mi, c0, c1, off) in bank:
                                nc.tensor.matmul(s[:, i, off:off + c1 - c0],
                                                 kT[:, pjt * P:(pjt + 1) * P],
                                                 q_s[:, c0:c1],
                                                 start=True, stop=True)
                        nc.scalar.activation(pt[:, 0:n, :], s[:, 0:n, :], ACTF.Exp,
                                             bias=0.0, scale=SCALE)
                        for i, bank in enumerate(items):
                            for (pjt, mi, c0, c1, off) in bank:
                                nc.vector.tensor_tensor(
                                    pt[:, i, off:off + c1 - c0],
                                    pt[:, i, off:off + c1 - c0],
                                    em2[:, mi, c0:c1], ALU.mult)
                        for i, bank in enumerate(items):
                            for (pjt, mi, c0, c1, off) in bank:
                                pvi += 1
                                pv(v_sb[:, pjt, :], pt[:, i, off:off + c1 - c0],
                                   c0, c1, pvi == npv)

                if post_cb is not None:
                    post_cb()
                # evict po -> f16, DMA-transpose, normalize into att
                ao = wk.tile([80, 512], F16, tag="ao")
                nc.vector.tensor_copy(ao[0:65, :], po[0:65, :])
                aot = wk.tile([P, TTH, 80], F16, tag="aot")
                nc.sync.dma_start_transpose(aot[:], ao[:])
                rec4 = wk.tile([P, TTH, 1], F32, tag="rec4")
                nc.vector.reciprocal(rec4[:], aot[:, :, 64:65])
                nc.vector.scalar_tensor_tensor(
                    att[:, TTH * tc:TTH * (tc + 1), h * HD:(h + 1) * HD],
                    aot[:, :, 0:HD], 1.0,
                    rec4[:].to_broadcast((P, TTH, HD)), ALU.mult, ALU.mult)

            def gmax_ag_a(c):
                """local amax + AllGather launch."""
                hsl = slice(TTH * c, TTH * (c + 1))
                for i in range(TTH):
                    tt = TTH * c + i
                    nc.vector.tensor_reduce(g_loc[:, tt:tt + 1], att[:, tt, :],
                                            AX.X, ALU.max, apply_absolute_value=True)
                nc.vector.tensor_scalar(g_loc[:, hsl], g_loc[:, hsl],
                                        1e-5, None, ALU.max)
                nc.scalar.dma_start(cc_in[c][:], g_loc[:, hsl])
                nc.gpsimd.collective_compute(
                    "AllGather", ALU.bypass,
                    replica_groups=[list(range(NCORES))],
                    ins=[cc_in[c].opt()], outs=[cc_out[c].opt()])

            def gmax_ag_b(c):
                """collect AllGather result (issued late: the gg load blocks
                the DVE queue until the collective lands)."""
                hsl = slice(TTH * c, TTH * (c + 1))
                gg = wk.tile([P, TTH, NCORES], F32, tag="gg")
                nc.sync.dma_start(gg[:], cc_out[c][:].rearrange("c p f -> p f c"))
                nc.vector.tensor_reduce(g_glob[:, hsl], gg[:], AX.X, ALU.max)

            def phaseD_tt(tt, ceng, eengs, pe_tr=False):
                rec = wk3.tile([P, 1], F32, tag="drec")
                nc.vector.reciprocal(rec[:], g_glob[:, tt:tt + 1])
                xi = wk3.tile([P, HPC * HD], I8, tag="dxi")
                nc.vector.tensor_scalar(xi[:], att[:, tt, :], rec[:], 127.0,
                                        ALU.mult, ALU.mult)
                xb = wk3.tile([P, HPC * HD], BF16, tag="dxb")
                copy_(ceng, xb[:], xi[:])
                xq2t = wk3.tile([P, 2, P], BF16, tag="dxq2t")
                if pe_tr:
                    ptr = pap.tile([P, 8, P], BF16, tag="tp")
                    for ko in range(2):
                        nc.tensor.transpose(ptr[:, ko, :], xb[:, ko * P:(ko + 1) * P],
                                            ident[:])
                    nc.vector.tensor_copy(xq2t[:], ptr[:, 0:2, :])
                else:
                    nc.sync.dma_start_transpose(xq2t[:], xb[:])
                ysc = wk3.tile([P, 1], F32, tag="dysc")
                nc.vector.tensor_tensor(ysc[:], g_glob[:, tt:tt + 1], osc[:], ALU.mult)
                ysb = wk3.tile([P, H], F16, tag="dy")
                s0 = scp.tile([P, G, 512], F32, tag="s")
                s1 = scp.tile([P, G, 512], F32, tag="s")
                for nch in range(4):
                    sl = slice(nch * 512, (nch + 1) * 512)
                    psy = (s0 if nch < 3 else s1)[:, nch % 3, :]
                    for ko in range(2):
                        nc.tensor.matmul(psy, xq2t[:, ko, :],
                                         wot[:, ko, sl],
                                         start=(ko == 0), stop=(ko == 1))
                    scale_(eengs[nch % len(eengs)], ysb[:, sl], psy, ysc[:])
                    if nch == 1:
                        nc.sync.dma_start(y_d[tt * P:(tt + 1) * P, 0:1024],
                                          ysb[:, 0:1024])
                nc.sync.dma_start(y_d[tt * P:(tt + 1) * P, 1024:2048],
                                  ysb[:, 1024:2048])

            # ---------------- schedule --------------------------------------
            # phase A chunk 0: copies on Pool so the Act queue stays short;
            # kbq projections sub-chunked so they start after x-tiles 0,1.
            mk('A-c0')
            phaseA_tile(0, nc.vector, nc.gpsimd, nc.scalar)
            phaseA_tile(1, nc.vector, nc.gpsimd, nc.scalar)
            mk('B0-kbq-sub0')
            phaseB_m1(0, 0, sub=0)
            phaseB_m1(1, 0, sub=0)
            phaseA_tile(2, nc.vector, nc.gpsimd, nc.scalar)
            phaseA_tile(3, nc.vector, nc.gpsimd, nc.scalar)
            inv_a_bounce(0)
            mk('B0-kbq-sub1')
            phaseB_m1(0, 0, sub=1)
            phaseB_m1(1, 0, sub=1)

            def b_rest_1(c):
                mk(f'b_rest1_{c}')
                phaseB_m1(4, c)          # k | v
                rope(kT, None, c, "k")
                v_transpose(c)
                phaseB_m1(2, c)          # q heads 0,1
                rope(qT, [0, 1], c, "q")

            def b_rest_2(c):
                mk(f'b_rest2_{c}')
                phaseB_m1(3, c)          # q heads 2,3
                rope(qT, [2, 3], c, "q")

            def b0_rest():
                b_rest_1(0)
                # fill the Act hole while the rope chain drains
                mk('a1-45')
                phaseA_tile(4, nc.vector, nc.scalar, nc.scalar)
                phaseA_tile(5, nc.vector, nc.scalar, nc.scalar)

            def b1_kbq():
                mk('b1_kbq')
                phaseB_m1(0, 1)
                phaseB_m1(1, 1)

            # issue x4..x7 loads (slots free as phase A consumed x0..x3)
            for tt in range(TTH, TT):
                t = xp.tile([P, H], F32, tag="x")
                nc.sync.dma_start(t[:], x_d[tt * P:(tt + 1) * P, :])
                xt[tt] = t
            nc.sync.dma_start(cos2[:], cos_d)
            nc.sync.dma_start(sin2[:], sin_d)
            nc.sync.dma_start(rmat[:], rmat_d)
            nc.sync.dma_start(em2[:], em2_d)
            wot = cst.tile([P, 2, H], BF16)
            nc.sync.dma_start(wot[:], wot_d.rearrange("(ko p) o -> p ko o", p=P))
            osc = cst.tile([P, 1], F32)
            nc.sync.dma_start(osc[:], osc_d)

            # C tc0: rest of B-c0 + A tiles 4,5 inside h0; A-c1 tiles 6,7
            # inside h1/h2 (copies on Pool, evicts on DVE: Act stays on exps)
            a1_sched = {1: [6, 7]}
            tc0_mids = {0: b0_rest}
            tc0_posts = {1: lambda: b_rest_2(0), 2: b1_kbq}
            for h in range(HPC):
                mk(f'C-tc0-h{h}')
                attn_head(h, 0, mid_cb=tc0_mids.get(h),
                          post_cb=tc0_posts.get(h))
                for tt in a1_sched.get(h, []):
                    phaseA_tile(tt, nc.gpsimd, nc.gpsimd, nc.vector)
                if h == 1:
                    inv_a_bounce(1)

            mk('gmax_a0')
            gmax_ag_a(0)

            # C tc1, with remaining B-c1 inside h0/h1
            tc1_mids = {0: lambda: b_rest_1(1)}
            tc1_posts = {1: lambda: b_rest_2(1)}
            for h in range(HPC):
                mk(f'C-tc1-h{h}')
                attn_head(h, 1, mid_cb=tc1_mids.get(h),
                          post_cb=tc1_posts.get(h))
            mk('gmax_a1')
            gmax_ag_a(1)
            gmax_ag_b(0)
            mk('D-c0')
            for tt in range(TTH):
                phaseD_tt(tt, nc.gpsimd, (nc.vector, nc.scalar), pe_tr=True)
            gmax_ag_b(1)

            # D half 1 (PE transposes: PE idle in the tail, lower latency)
            mk('D-c1')
            for tt in range(TTH, TT):
                phaseD_tt(tt, nc.scalar, (nc.scalar, nc.vector), pe_tr=True)

    nc.compile()
    return nc


def _quant_w(w):
    ws = np.float32(1.0) / np.float32(np.clip(np.mean(np.abs(w)), 1e-5, None))
    wq = np.clip(np.round(w.astype(np.float32) * ws), -1.0, 1.0)
    return wq, ws


def _mask_structure(mask):
    """mask: [Q, Q] additive. Returns (cfg, em2 host tensor [P, NM, 512] f16, NM)."""
    em = np.exp(mask.astype(np.float32)).T.astype(np.float16)  # [keys, queries]
    cfg = []
    em_blocks = []
    mi = 0
    for tc in range(2):
        fulls, maskeds = [], []
        for pjt in range(TT):
            blk = em[pjt * P:(pjt + 1) * P, tc * 512:(tc + 1) * 512]
            if not blk.any():
                continue
            if (blk == np.float16(1.0)).all():
                fulls.append(pjt)
            else:
                cols = np.nonzero(blk.any(axis=0))[0]
                c0, c1 = int(cols[0]), int(cols[-1]) + 1
                maskeds.append((pjt, mi, c0, c1))
                em_blocks.append(blk)
                mi += 1
        cfg.append((tuple(fulls), tuple(maskeds)))
    nm = len(em_blocks)
    em2 = (np.stack(em_blocks, axis=1) if nm
           else np.zeros((P, 1, 512), np.float16))
    return tuple(cfg), np.ascontiguousarray(em2), nm


def _prep_inputs(inputs):
    hs = np.ascontiguousarray(np.asarray(inputs["hidden_states"], np.float32)[0])
    mask = np.asarray(inputs["attention_mask"], np.float32)[0, 0]
    kbk = np.asarray(inputs["kb_keys"], np.float32)[0]
    kbvv = np.asarray(inputs["kb_values"], np.float32)[0]
    pos = np.asarray(inputs["position_ids"])[0].astype(np.float32)

    wq_i, wsq = _quant_w(np.asarray(inputs["Wq"], np.float32))
    wk_i, wsk = _quant_w(np.asarray(inputs["Wk"], np.float32))
    wv_i, wsv = _quant_w(np.asarray(inputs["Wv"], np.float32))
    wo_i, wso = _quant_w(np.asarray(inputs["Wo"], np.float32))
    wqn_i, wsqn = _quant_w(np.asarray(inputs["Wq_new"], np.float32))

    inv_freq = 1.0 / (10000.0 ** (np.arange(0, HD, 2, dtype=np.float32) / HD))
    freqs = pos[None, :] * inv_freq[:, None]          # [32, Q]
    cos2 = np.concatenate([np.cos(freqs), np.cos(freqs)], 0).astype(np.float16)
    sin2 = np.concatenate([np.sin(freqs), np.sin(freqs)], 0).astype(np.float16)
    # rotate_half as a signed permutation matrix (lhsT = R^T)
    rmat = np.zeros((HD, HD), np.float16)
    for dd in range(32):
        rmat[dd, dd + 32] = -1.0      # out[d] = -in[d+32]
        rmat[dd + 32, dd] = 1.0       # out[d+32] = in[d]
    rmat_t = np.ascontiguousarray(rmat.T)

    cfg, em2, nm = _mask_structure(mask)

    in_maps = []
    for c in range(NCORES):
        qsl = slice(HPC * HD * c, HPC * HD * (c + 1))
        ksl = slice(HD * c, HD * (c + 1))
        # order: kbq(256) | q(256) | k(64) | v(64)
        w1 = np.concatenate([wqn_i[qsl], wq_i[qsl], wk_i[ksl], wv_i[ksl]], 0)
        wsvec = np.concatenate([
            np.full(256, 1.0 / wsqn, np.float32),
            np.full(256, 1.0 / wsq, np.float32),
            np.full(64, 1.0 / wsk, np.float32),
            np.full(64, 1.0 / wsv, np.float32)])
        kbkt = np.ascontiguousarray(
            kbk[HPC * c:HPC * (c + 1)].transpose(2, 0, 1)).astype(np.float16)
        kbva = np.concatenate(
            [kbvv[HPC * c:HPC * (c + 1)],
             np.ones((HPC, KB, 1), np.float32)], -1).astype(np.float16)
        kbva = np.ascontiguousarray(
            kbva.reshape(HPC, NJT, P, 65).transpose(2, 0, 1, 3))
        wot = np.ascontiguousarray(wo_i[:, qsl].T).astype(ml_dtypes.bfloat16)
        w1t = np.ascontiguousarray(
            w1.T.reshape(KO, P, 640).transpose(1, 0, 2)).astype(
                ml_dtypes.float8_e4m3)
        in_maps.append({
            "x": hs,
            "w1t": w1t,
            "wsvec": wsvec,
            "cos2": np.ascontiguousarray(cos2),
            "sin2": np.ascontiguousarray(sin2),
            "kbkt": kbkt,
            "kbv": np.ascontiguousarray(kbva),
            "em2": em2,
            "rmat": rmat_t,
            "wot": wot,
            "oscale": np.full((P, 1), 1.0 / (127.0 * wso), np.float32),
        })
    return in_maps, cfg, nm


def kernel(**inputs) -> np.ndarray:
    in_maps, cfg, nm = _prep_inputs(inputs)
    key = (cfg, nm)
    if key not in _CACHE:
        _CACHE[key] = _build(cfg, nm)
    nc = _CACHE[key]
    res = bass_utils.run_bass_kernel_spmd(nc, in_maps, core_ids=list(range(NCORES)))
    y = np.zeros((Q, H), np.float64)
    for c in range(NCORES):
        y += res.results[c]["y"].astype(np.float64)
    return y.astype(np.float32)[None]


# revision 61
# speedup vs baseline: 1.0276x; 1.0276x over previous
"""KBLaM BitNet attention on 8 Trainium2 NeuronCores (tensor-parallel over heads).

Core c owns q-heads 4c..4c+3, kv-head c, kb heads 4c..4c+3, and the matching
input-dim slice of Wo. Each core returns a partial o_proj output in f16; the
host sums.

Numerics: BitLinear projections are exact (integer activations / ternary
weights in bf16, fp32 PSUM accumulation). Attention (QK^T, probs, PV) runs in
f16 with fp32 PSUM accumulation; RoPE runs in f16. The o_proj activation
quantization uses a round-half-even saturating int8 cast, identical to
clip(round(x*a), -128, 127). Two pipelined AllGathers (one per 512-token half)
provide the global per-token amax for that quantization.
"""
import sys
if "/opt/trn_rl_repo" not in sys.path:
    sys.path.insert(0, "/opt/trn_rl_repo")
import numpy as np
import ml_dtypes

import concourse.mybir as mybir
import concourse.tile as tile
from concourse import bacc
from concourse import bass_utils
from concourse.masks import make_identity

F32 = mybir.dt.float32
F16 = mybir.dt.float16
BF16 = mybir.dt.bfloat16
F8 = mybir.dt.float8e4
I8 = mybir.dt.int8
ALU = mybir.AluOpType
ACTF = mybir.ActivationFunctionType
AX = mybir.AxisListType

B, Q, H = 1, 1024, 2048
NH, NKV, HD = 32, 8, 64
KB = 2048
NCORES = 8
HPC = NH // NCORES            # 4 q heads per core
P = 128
TT = Q // P                   # 8 token tiles
TTH = TT // 2                 # 4 token tiles per half
KO = H // P                   # 16 hidden k-tiles
M1 = 5                        # proj out tiles: kbq 256 | q 256 | (k 64 + v 64)
NJT = KB // P                 # 16 kb j-tiles
G = 3                         # scores psum group size (j-tiles per exp)
SCALE = 0.125                 # 1/sqrt(HD)
KB_BIAS = float(np.log(4096.0) - np.log(float(KB)))

_CACHE = {}
_MARKS = []


def _grouped(blocks, g):
    return [blocks[i:i + g] for i in range(0, len(blocks), g)]


def _build(MASK_CFG, NM):
    """MASK_CFG: per tc in (0,1): (fulls tuple, maskeds tuple of (pjt, mi, c0, c1)).
    NM = total number of masked em blocks staged in em2."""
    nc = bacc.Bacc("TRN2", target_bir_lowering=False, debug=False, num_devices=NCORES)

    x_d = nc.dram_tensor("x", [Q, H], F32, kind="ExternalInput").ap()
    w1t_d = nc.dram_tensor("w1t", [P, KO, 640], F8, kind="ExternalInput").ap()
    wsvec_d = nc.dram_tensor("wsvec", [640], F32, kind="ExternalInput").ap()
    cos_d = nc.dram_tensor("cos2", [HD, Q], F16, kind="ExternalInput").ap()
    sin_d = nc.dram_tensor("sin2", [HD, Q], F16, kind="ExternalInput").ap()
    kbkt_d = nc.dram_tensor("kbkt", [HD, HPC, KB], F16, kind="ExternalInput").ap()
    kbv_d = nc.dram_tensor("kbv", [P, HPC, NJT, 65], F16, kind="ExternalInput").ap()
    em2_d = nc.dram_tensor("em2", [P, max(NM, 1), 512], F16, kind="ExternalInput").ap()
    rmat_d = nc.dram_tensor("rmat", [HD, HD], F16, kind="ExternalInput").ap()
    wot_d = nc.dram_tensor("wot", [HPC * HD, H], BF16, kind="ExternalInput").ap()
    osc_d = nc.dram_tensor("oscale", [P, 1], F32, kind="ExternalInput").ap()
    y_d = nc.dram_tensor("y", [Q, H], F16, kind="ExternalOutput").ap()

    def copy_(eng, out, in_):
        if eng is nc.scalar:
            nc.scalar.copy(out, in_)
        else:
            eng.tensor_copy(out, in_)

    def scale_(eng, out, in_, sc):
        if eng is nc.scalar:
            nc.scalar.mul(out, in_, sc)
        else:
            eng.tensor_scalar(out, in_, sc, None, ALU.mult)

    def mk(label):
        _MARKS.append((label, nc.next_id()))

    with tile.TileContext(nc) as tc_:
        with tc_.tile_pool(name="cst", bufs=1) as cst, \
             tc_.tile_pool(name="xp", bufs=3) as xp, \
             tc_.tile_pool(name="sc", bufs=2, space="PSUM") as scp, \
             tc_.tile_pool(name="pop", bufs=1, space="PSUM") as pop, \
             tc_.tile_pool(name="pap", bufs=1, space="PSUM") as pap, \
             tc_.tile_pool(name="wk", bufs=2) as wk, \
             tc_.tile_pool(name="wk3", bufs=3) as wk3, \
             tc_.tile_pool(name="wk4", bufs=4) as wk4, \
             tc_.tile_pool(name="dram", bufs=1, space="DRAM") as dram:

            # ---------------- DMA prologue (ordered for earliest use) -------
            xt = {}
            for tt in range(TTH):
                t = xp.tile([P, H], F32, tag="x")
                nc.sync.dma_start(t[:], x_d[tt * P:(tt + 1) * P, :])
                xt[tt] = t
            w1t = cst.tile([P, KO, 640], F8)
            nc.sync.dma_start(w1t[:], w1t_d)
            wspp = cst.tile([P, M1], F32)
            nc.sync.dma_start(wspp[:], wsvec_d.rearrange("(m p) -> p m", p=P))
            kbkt = cst.tile([HD, HPC, KB], F16)
            nc.sync.dma_start(kbkt[:], kbkt_d)
            kbv = cst.tile([P, HPC, NJT, 65], F16)
            nc.sync.dma_start(kbv[:], kbv_d)
            cos2 = cst.tile([HD, Q], F16)
            sin2 = cst.tile([HD, Q], F16)
            em2 = cst.tile([P, max(NM, 1), 512], F16)
            rmat = cst.tile([HD, HD], F16)

            # ---------------- resident constants / buffers ------------------
            kbias = cst.tile([P, 1], F32)
            nc.vector.memset(kbias[:], KB_BIAS)
            ident = cst.tile([P, P], BF16)
            make_identity(nc, ident)

            inv_a_cols = cst.tile([P, TT], F32)
            inv_a_dram = dram.tile([Q], F32)
            inv_ab = cst.tile([P, Q], F32)
            xqT = cst.tile([P, KO, Q], BF16)
            qT = cst.tile([HD, HPC, Q], F16)
            kbqT = cst.tile([HD, HPC, Q], F16)
            kT = cst.tile([HD, Q], F16)
            vTf = cst.tile([HD, Q], F16)
            v_sb = cst.tile([P, TT, 65], F16)
            nc.vector.memset(v_sb[:], 1.0)
            att = cst.tile([P, TT, HPC * HD], F32)
            g_loc = cst.tile([P, TT], F32)
            g_glob = cst.tile([P, TT], F32)
            cc_in = [dram.tile([P, TTH], F32, name=f"ci{c}") for c in range(2)]
            cc_out = [dram.tile([NCORES, P, TTH], F32, name=f"co{c}") for c in range(2)]

            # ---------------- phase helpers --------------------------------
            def phaseA_tile(tt, qeng, ceng, eeng):
                """quantize x tile tt, transpose into xqT."""
                x_ = xt[tt]
                m = wk.tile([P, 1], F32, tag="am")
                nc.vector.tensor_reduce(m[:], x_[:], AX.X, ALU.max,
                                        apply_absolute_value=True)
                nc.vector.tensor_scalar(m[:], m[:], 1e-5, None, ALU.max)
                rec = wk.tile([P, 1], F32, tag="arec")
                nc.vector.reciprocal(rec[:], m[:])
                a_col = wk.tile([P, 1], F32, tag="acol")
                nc.vector.tensor_scalar(a_col[:], rec[:], 127.0, None, ALU.mult)
                nc.vector.tensor_scalar(inv_a_cols[:, tt:tt + 1], m[:],
                                        1.0 / 127.0, None, ALU.mult)
                xi = wk.tile([P, H], I8, tag="axi")
                nc.vector.tensor_scalar(xi[:], x_[:], a_col[:], None, ALU.mult)
                xq = wk.tile([P, H], BF16, tag="axq")
                copy_(ceng, xq[:], xi[:])
                for g in range(2):
                    pt = pap.tile([P, 8, P], BF16, tag="tp")
                    for i in range(8):
                        ko = 8 * g + i
                        nc.tensor.transpose(pt[:, i, :],
                                            xq[:, ko * P:(ko + 1) * P], ident[:])
                    copy_(eeng, xqT[:, 8 * g:8 * g + 8, tt * P:(tt + 1) * P], pt[:])

            def inv_a_bounce(c):
                sl = slice(512 * c, 512 * (c + 1))
                nc.sync.dma_start(
                    inv_a_dram[:].rearrange("(o p) -> p o", p=P)[:, TTH * c:TTH * (c + 1)],
                    inv_a_cols[:, TTH * c:TTH * (c + 1)])
                nc.sync.dma_start(
                    inv_ab[:, sl],
                    inv_a_dram[sl].unsqueeze(0).partition_broadcast(P))

            b_slots = {}

            def phaseB_m1(m1, c, sub=None):
                """projection group m1 for token chunk c + dequant to f16.
                sub=0/1 runs a 256-token half (dequant after sub=1)."""
                sl = slice(512 * c, 512 * (c + 1))
                if sub is None:
                    s = scp.tile([P, G, 512], F32, tag="s")
                    ps = s[:, 0, :]
                    msl = sl
                else:
                    if sub == 0:
                        b_slots[(m1, c)] = scp.tile([P, G, 512], F32, tag="s",
                                                    name=f"bs{m1}_{c}")
                    s = b_slots[(m1, c)]
                    ps = s[:, 0, sub * 256:(sub + 1) * 256]
                    msl = slice(512 * c + sub * 256, 512 * c + (sub + 1) * 256)
                for ko in range(KO):
                    nc.tensor.matmul(ps, w1t[:, ko, m1 * P:(m1 + 1) * P],
                                     xqT[:, ko, msl],
                                     start=(ko == 0), stop=(ko == KO - 1))
                if sub == 0:
                    return
                ps = s[:, 0, :]
                if m1 < 2:      # kbq heads 2*m1, 2*m1+1 (no rope)
                    dsts = (kbqT[:, 2 * m1, sl], kbqT[:, 2 * m1 + 1, sl])
                elif m1 < 4:    # q heads (rope applied in-place later)
                    dsts = (qT[:, 2 * (m1 - 2), sl], qT[:, 2 * (m1 - 2) + 1, sl])
                else:           # k | v
                    dsts = (kT[:, sl], vTf[:, sl])
                nc.vector.scalar_tensor_tensor(
                    dsts[0], ps[:HD], wspp[:HD, m1:m1 + 1],
                    inv_ab[:HD, sl], ALU.mult, ALU.mult)
                nc.vector.scalar_tensor_tensor(
                    dsts[1], ps[HD:], wspp[HD:, m1:m1 + 1],
                    inv_ab[HD:, sl], ALU.mult, ALU.mult)

            def rope(dst, heads, c, tag):
                """in-place RoPE on dst[HD, (h,) 512c:512c+512], f16.
                heads=None: single-vector (k). rotate_half is a fixed 64x64
                signed permutation, computed on the PE (low latency)."""
                sl = slice(512 * c, 512 * (c + 1))
                hs = [0] if heads is None else heads
                nh = len(hs)
                dsl = [dst[:, sl].unsqueeze(1)[:, 0, :] if heads is None
                       else dst[:, h, sl] for h in hs]
                srot = []
                for i in range(nh):
                    if i % G == 0:
                        s = scp.tile([P, G, 512], F32, tag="s",
                                     name=f"rp{tag}{c}{i}")
                    srot.append(s[0:HD, i % G, :])
                    nc.tensor.matmul(srot[i], rmat[:], dsl[i],
                                     start=True, stop=True)
                t2 = wk.tile([HD, 2 if tag == "q" else 1, 512], F16,
                             tag=f"sw{tag}")
                t1 = wk.tile([HD, 2 if tag == "q" else 1, 512], F16,
                             tag=f"rt{tag}")
                for i in range(nh):
                    nc.vector.tensor_tensor(t2[:, i, :], srot[i],
                                            sin2[:, sl], ALU.mult)
                    nc.vector.tensor_tensor(t1[:, i, :], dsl[i],
                                            cos2[:, sl], ALU.mult)
                    nc.vector.tensor_tensor(dsl[i], t1[:, i, :], t2[:, i, :],
                                            ALU.add)

            def v_transpose(c):
                vt = wk.tile([P, TTH, HD], F16, tag="vt")
                nc.sync.dma_start_transpose(vt[:], vTf[:, 512 * c:512 * (c + 1)])
                nc.vector.tensor_copy(v_sb[:, TTH * c:TTH * (c + 1), 0:HD], vt[:])

            def attn_head(h, tc, mid_cb=None, post_cb=None):
                """one head, one 512-query chunk."""
                sl = slice(512 * tc, 512 * (tc + 1))
                kbq_s = kbqT[:, h, sl]
                q_s = qT[:, h, sl]
                po = pop.tile([P, 512], F32, tag="po")
                state = {"started": False}

                def pv(lhsT, pt_ap, c0, c1, last):
                    nc.tensor.matmul(po[0:65, c0:c1], lhsT, pt_ap,
                                     start=(not state["started"]), stop=last,
                                     skip_group_check=True)
                    state["started"] = True

                fulls, maskeds = MASK_CFG[tc]
                # bank-pack masked blocks: first-fit-decreasing into 512-wide
                # PSUM banks so one exp covers several partial blocks
                banks = []   # list of [used_width, [(pjt, mi, c0, c1, off)]]
                for (pjt, mi, c0, c1) in sorted(maskeds, key=lambda t: t[2] - t[3]):
                    w = c1 - c0
                    for b in banks:
                        if b[0] + w <= 512:
                            b[1].append((pjt, mi, c0, c1, b[0]))
                            b[0] += w
                            break
                    else:
                        banks.append([w, [(pjt, mi, c0, c1, 0)]])
                groups = [("kb", ch) for ch in _grouped(list(range(NJT)), G)]
                groups += [("pf", ch) for ch in _grouped(list(fulls), G)]
                groups += [("pm", ch) for ch in
                           _grouped([b[1] for b in banks], G)]
                npv = sum(sum(len(it) for it in g[1]) if g[0] == "pm"
                          else len(g[1]) for g in groups)
                pvi = 0
                nkb = len(_grouped(list(range(NJT)), G))
                for gi, (kind, items) in enumerate(groups):
                    if gi == nkb and mid_cb is not None:
                        mid_cb()
                    n = len(items)
                    s = scp.tile([P, G, 512], F32, tag="s")
                    pt = wk4.tile([P, G, 512], F16, tag="pt")
                    if kind == "kb":
                        for i, jt in enumerate(items):
                            nc.tensor.matmul(s[:, i, :],
                                             kbkt[:, h, jt * P:(jt + 1) * P],
                                             kbq_s, start=True, stop=True)
                        nc.scalar.activation(pt[:, 0:n, :], s[:, 0:n, :], ACTF.Exp,
                                             bias=kbias[:], scale=SCALE)
                        for i, jt in enumerate(items):
                            pvi += 1
                            pv(kbv[:, h, jt, :], pt[:, i, :], 0, 512, pvi == npv)
                    elif kind == "pf":
                        for i, pjt in enumerate(items):
                            nc.tensor.matmul(s[:, i, :],
                                             kT[:, pjt * P:(pjt + 1) * P],
                                             q_s, start=True, stop=True)
                        nc.scalar.activation(pt[:, 0:n, :], s[:, 0:n, :], ACTF.Exp,
                                             bias=0.0, scale=SCALE)
                        for i, pjt in enumerate(items):
                            pvi += 1
                            pv(v_sb[:, pjt, :], pt[:, i, :], 0, 512, pvi == npv)
                    else:  # masked: items = list of banks of placed blocks
                        for i, bank in enumerate(items):
                            for (pjt, mi, c0, c1, off) in bank:
                                nc.tensor.matmul(s[:, i, off:off + c1 - c0],
                                                 kT[:, pjt * P:(pjt + 1) * P],
                                                 q_s[:, c0:c1],
                                                 start=True, stop=True)
                        nc.scalar.activation(pt[:, 0:n, :], s[:, 0:n, :], ACTF.Exp,
                                             bias=0.0, scale=SCALE)
                        for i, bank in enumerate(items):
                            for (pjt, mi, c0, c1, off) in bank:
                                nc.vector.tensor_tensor(
                                    pt[:, i, off:off + c1 - c0],
                                    pt[:, i, off:off + c1 - c0],
                                    em2[:, mi, c0:c1], ALU.mult)
                        for i, bank in enumerate(items):
                            for (pjt, mi, c0, c1, off) in bank:
                                pvi += 1
                                pv(v_sb[:, pjt, :], pt[:, i, off:off + c1 - c0],
                                   c0, c1, pvi == npv)

                if post_cb is not None:
                    post_cb()
                # evict po -> f16, DMA-transpose, normalize into att
                ao = wk.tile([80, 512], F16, tag="ao")
                nc.vector.tensor_copy(ao[0:65, :], po[0:65, :])
                aot = wk.tile([P, TTH, 80], F16, tag="aot")
                nc.sync.dma_start_transpose(aot[:], ao[:])
                rec4 = wk.tile([P, TTH, 1], F32, tag="rec4")
                nc.vector.reciprocal(rec4[:], aot[:, :, 64:65])
                nc.vector.scalar_tensor_tensor(
                    att[:, TTH * tc:TTH * (tc + 1), h * HD:(h + 1) * HD],
                    aot[:, :, 0:HD], 1.0,
                    rec4[:].to_broadcast((P, TTH, HD)), ALU.mult, ALU.mult)

            def gmax_ag_a(c):
                """local amax + AllGather launch."""
                hsl = slice(TTH * c, TTH * (c + 1))
                for i in range(TTH):
                    tt = TTH * c + i
                    nc.vector.tensor_reduce(g_loc[:, tt:tt + 1], att[:, tt, :],
                                            AX.X, ALU.max, apply_absolute_value=True)
                nc.vector.tensor_scalar(g_loc[:, hsl], g_loc[:, hsl],
                                        1e-5, None, ALU.max)
                nc.scalar.dma_start(cc_in[c][:], g_loc[:, hsl])
                nc.gpsimd.collective_compute(
                    "AllGather", ALU.bypass,
                    replica_groups=[list(range(NCORES))],
                    ins=[cc_in[c].opt()], outs=[cc_out[c].opt()])

            def gmax_ag_b(c):
                """collect AllGather result (issued late: the gg load blocks
                the DVE queue until the collective lands)."""
                hsl = slice(TTH * c, TTH * (c + 1))
                gg = wk.tile([P, TTH, NCORES], F32, tag="gg")
                nc.sync.dma_start(gg[:], cc_out[c][:].rearrange("c p f -> p f c"))
                nc.vector.tensor_reduce(g_glob[:, hsl], gg[:], AX.X, ALU.max)

            def phaseD_tt(tt, ceng, eengs, pe_tr=False):
                rec = wk3.tile([P, 1], F32, tag="drec")
                nc.vector.reciprocal(rec[:], g_glob[:, tt:tt + 1])
                xi = wk3.tile([P, HPC * HD], I8, tag="dxi")
                nc.vector.tensor_scalar(xi[:], att[:, tt, :], rec[:], 127.0,
                                        ALU.mult, ALU.mult)
                xb = wk3.tile([P, HPC * HD], BF16, tag="dxb")
                copy_(ceng, xb[:], xi[:])
                xq2t = wk3.tile([P, 2, P], BF16, tag="dxq2t")
                if pe_tr:
                    ptr = pap.tile([P, 8, P], BF16, tag="tp")
                    for ko in range(2):
                        nc.tensor.transpose(ptr[:, ko, :], xb[:, ko * P:(ko + 1) * P],
                                            ident[:])
                    nc.vector.tensor_copy(xq2t[:], ptr[:, 0:2, :])
                else:
                    nc.sync.dma_start_transpose(xq2t[:], xb[:])
                ysc = wk3.tile([P, 1], F32, tag="dysc")
                nc.vector.tensor_tensor(ysc[:], g_glob[:, tt:tt + 1], osc[:], ALU.mult)
                ysb = wk3.tile([P, H], F16, tag="dy")
                s0 = scp.tile([P, G, 512], F32, tag="s")
                s1 = scp.tile([P, G, 512], F32, tag="s")
                for nch in range(4):
                    sl = slice(nch * 512, (nch + 1) * 512)
                    psy = (s0 if nch < 3 else s1)[:, nch % 3, :]
                    for ko in range(2):
                        nc.tensor.matmul(psy, xq2t[:, ko, :],
                                         wot[:, ko, sl],
                                         start=(ko == 0), stop=(ko == 1))
                    scale_(eengs[nch % len(eengs)], ysb[:, sl], psy, ysc[:])
                    if nch == 1:
                        nc.sync.dma_start(y_d[tt * P:(tt + 1) * P, 0:1024],
                                          ysb[:, 0:1024])
                nc.sync.dma_start(y_d[tt * P:(tt + 1) * P, 1024:2048],
                                  ysb[:, 1024:2048])

            # ---------------- schedule --------------------------------------
            # phase A chunk 0: copies on Pool so the Act queue stays short;
            # kbq projections sub-chunked so they start after x-tiles 0,1.
            mk('A-c0')
            phaseA_tile(0, nc.vector, nc.gpsimd, nc.scalar)
            phaseA_tile(1, nc.vector, nc.gpsimd, nc.scalar)
            mk('B0-kbq-sub0')
            phaseB_m1(0, 0, sub=0)
            phaseB_m1(1, 0, sub=0)
            phaseA_tile(2, nc.vector, nc.gpsimd, nc.scalar)
            phaseA_tile(3, nc.vector, nc.gpsimd, nc.scalar)
            inv_a_bounce(0)
            mk('B0-kbq-sub1')
            phaseB_m1(0, 0, sub=1)
            phaseB_m1(1, 0, sub=1)

            def b_rest_1(c):
                mk(f'b_rest1_{c}')
                phaseB_m1(4, c)          # k | v
                rope(kT, None, c, "k")
                v_transpose(c)
                phaseB_m1(2, c)          # q heads 0,1
                rope(qT, [0, 1], c, "q")

            def b_rest_2(c):
                mk(f'b_rest2_{c}')
                phaseB_m1(3, c)          # q heads 2,3
                rope(qT, [2, 3], c, "q")

            def b0_rest():
                b_rest_1(0)
                # fill the Act hole while the rope chain drains
                mk('a1-45')
                phaseA_tile(4, nc.vector, nc.scalar, nc.scalar)
                phaseA_tile(5, nc.vector, nc.scalar, nc.scalar)

            def b1_kbq():
                mk('b1_kbq')
                phaseB_m1(0, 1)

            # issue x4..x7 loads (slots free as phase A consumed x0..x3)
            for tt in range(TTH, TT):
                t = xp.tile([P, H], F32, tag="x")
                nc.sync.dma_start(t[:], x_d[tt * P:(tt + 1) * P, :])
                xt[tt] = t
            nc.sync.dma_start(cos2[:], cos_d)
            nc.sync.dma_start(sin2[:], sin_d)
            nc.sync.dma_start(rmat[:], rmat_d)
            nc.sync.dma_start(em2[:], em2_d)
            wot = cst.tile([P, 2, H], BF16)
            nc.sync.dma_start(wot[:], wot_d.rearrange("(ko p) o -> p ko o", p=P))
            osc = cst.tile([P, 1], F32)
            nc.sync.dma_start(osc[:], osc_d)

            # C tc0: rest of B-c0 + A tiles 4,5 inside h0; A-c1 tiles 6,7
            # inside h1/h2 (copies on Pool, evicts on DVE: Act stays on exps)
            a1_sched = {1: [6], 2: [7]}
            tc0_mids = {0: b0_rest}
            tc0_posts = {1: lambda: b_rest_2(0)}
            for h in range(HPC):
                mk(f'C-tc0-h{h}')
                attn_head(h, 0, mid_cb=tc0_mids.get(h),
                          post_cb=tc0_posts.get(h))
                for tt in a1_sched.get(h, []):
                    phaseA_tile(tt, nc.gpsimd, nc.gpsimd, nc.vector)
                if h == 2:
                    inv_a_bounce(1)

            mk('gmax_a0')
            gmax_ag_a(0)
            b1_kbq()

            # C tc1, with remaining B-c1 inside h0/h1
            def b1_mid0():
                phaseB_m1(1, 1)
                b_rest_1(1)

            tc1_mids = {0: b1_mid0}
            tc1_posts = {1: lambda: b_rest_2(1)}
            for h in range(HPC):
                mk(f'C-tc1-h{h}')
                attn_head(h, 1, mid_cb=tc1_mids.get(h),
                          post_cb=tc1_posts.get(h))
            mk('gmax_a1')
            gmax_ag_a(1)
            gmax_ag_b(0)
            mk('D-c0')
            for tt in range(TTH):
                phaseD_tt(tt, nc.gpsimd, (nc.vector, nc.scalar), pe_tr=True)
            gmax_ag_b(1)

            # D half 1 (PE transposes: PE idle in the tail, lower latency)
            mk('D-c1')
            for tt in range(TTH, TT):
                phaseD_tt(tt, nc.scalar, (nc.scalar, nc.vector), pe_tr=True)

    nc.compile()
    return nc


def _quant_w(w):
    ws = np.float32(1.0) / np.float32(np.clip(np.mean(np.abs(w)), 1e-5, None))
    wq = np.clip(np.round(w.astype(np.float32) * ws), -1.0, 1.0)
    return wq, ws


def _mask_structure(mask):
    """mask: [Q, Q] additive. Returns (cfg, em2 host tensor [P, NM, 512] f16, NM)."""
    em = np.exp(mask.astype(np.float32)).T.astype(np.float16)  # [keys, queries]
    cfg = []
    em_blocks = []
    mi = 0
    for tc in range(2):
        fulls, maskeds = [], []
        for pjt in range(TT):
            blk = em[pjt * P:(pjt + 1) * P, tc * 512:(tc + 1) * 512]
            if not blk.any():
                continue
            if (blk == np.float16(1.0)).all():
                fulls.append(pjt)
            else:
                cols = np.nonzero(blk.any(axis=0))[0]
                c0, c1 = int(cols[0]), int(cols[-1]) + 1
                maskeds.append((pjt, mi, c0, c1))
                em_blocks.append(blk)
                mi += 1
        cfg.append((tuple(fulls), tuple(maskeds)))
    nm = len(em_blocks)
    em2 = (np.stack(em_blocks, axis=1) if nm
           else np.zeros((P, 1, 512), np.float16))
    return tuple(cfg), np.ascontiguousarray(em2), nm


def _prep_inputs(inputs):
    hs = np.ascontiguousarray(np.asarray(inputs["hidden_states"], np.float32)[0])
    mask = np.asarray(inputs["attention_mask"], np.float32)[0, 0]
    kbk = np.asarray(inputs["kb_keys"], np.float32)[0]
    kbvv = np.asarray(inputs["kb_values"], np.float32)[0]
    pos = np.asarray(inputs["position_ids"])[0].astype(np.float32)

    wq_i, wsq = _quant_w(np.asarray(inputs["Wq"], np.float32))
    wk_i, wsk = _quant_w(np.asarray(inputs["Wk"], np.float32))
    wv_i, wsv = _quant_w(np.asarray(inputs["Wv"], np.float32))
    wo_i, wso = _quant_w(np.asarray(inputs["Wo"], np.float32))
    wqn_i, wsqn = _quant_w(np.asarray(inputs["Wq_new"], np.float32))

    inv_freq = 1.0 / (10000.0 ** (np.arange(0, HD, 2, dtype=np.float32) / HD))
    freqs = pos[None, :] * inv_freq[:, None]          # [32, Q]
    cos2 = np.concatenate([np.cos(freqs), np.cos(freqs)], 0).astype(np.float16)
    sin2 = np.concatenate([np.sin(freqs), np.sin(freqs)], 0).astype(np.float16)
    # rotate_half as a signed permutation matrix (lhsT = R^T)
    rmat = np.zeros((HD, HD), np.float16)
    for dd in range(32):
        rmat[dd, dd + 32] = -1.0      # out[d] = -in[d+32]
        rmat[dd + 32, dd] = 1.0       # out[d+32] = in[d]
    rmat_t = np.ascontiguousarray(rmat.T)

    cfg, em2, nm = _mask_structure(mask)

    in_maps = []
    for c in range(NCORES):
        qsl = slice(HPC * HD * c, HPC * HD * (c + 1))
        ksl = slice(HD * c, HD * (c + 1))
        # order: kbq(256) | q(256) | k(64) | v(64)
        w1 = np.concatenate([wqn_i[qsl], wq_i[qsl], wk_i[ksl], wv_i[ksl]], 0)
        wsvec = np.concatenate([
            np.full(256, 1.0 / wsqn, np.float32),
            np.full(256, 1.0 / wsq, np.float32),
            np.full(64, 1.0 / wsk, np.float32),
            np.full(64, 1.0 / wsv, np.float32)])
        kbkt = np.ascontiguousarray(
            kbk[HPC * c:HPC * (c + 1)].transpose(2, 0, 1)).astype(np.float16)
        kbva = np.concatenate(
            [kbvv[HPC * c:HPC * (c + 1)],
             np.ones((HPC, KB, 1), np.float32)], -1).astype(np.float16)
        kbva = np.ascontiguousarray(
            kbva.reshape(HPC, NJT, P, 65).transpose(2, 0, 1, 3))
        wot = np.ascontiguousarray(wo_i[:, qsl].T).astype(ml_dtypes.bfloat16)
        w1t = np.ascontiguousarray(
            w1.T.reshape(KO, P, 640).transpose(1, 0, 2)).astype(
                ml_dtypes.float8_e4m3)
        in_maps.append({
            "x": hs,
            "w1t": w1t,
            "wsvec": wsvec,
            "cos2": np.ascontiguousarray(cos2),
            "sin2": np.ascontiguousarray(sin2),
            "kbkt": kbkt,
            "kbv": np.ascontiguousarray(kbva),
            "em2": em2,
            "rmat": rmat_t,
            "wot": wot,
            "oscale": np.full((P, 1), 1.0 / (127.0 * wso), np.float32),
        })
    return in_maps, cfg, nm


def kernel(**inputs) -> np.ndarray:
    in_maps, cfg, nm = _prep_inputs(inputs)
    key = (cfg, nm)
    if key not in _CACHE:
        _CACHE[key] = _build(cfg, nm)
    nc = _CACHE[key]
    res = bass_utils.run_bass_kernel_spmd(nc, in_maps, core_ids=list(range(NCORES)))
    y = np.zeros((Q, H), np.float64)
    for c in range(NCORES):
        y += res.results[c]["y"].astype(np.float64)
    return y.astype(np.float32)[None]
